# revision 13
# baseline (speedup 1.0000x reference)
"""Trainium2 Bass kernel for CustomizeLSTMCell (fused 4-matmul LSTM-like cell).

Math (per token row x of N=100000, H=150):
    pre    = s_in @ W_in + s_out @ W_out + h_in @ U_in + h_out @ U_out
    gate   = sigmoid(pre)
    cell   = gate * last_c + gate * gate = gate * (last_c + gate)
    hidden = gate * tanh(cell)
returns (hidden, cell)

Strategy: data-parallel over tokens across 8 cores (12500 rows/core, padded
to 12544 = 12*1024 + 256). Feature-major (transposed) on chip with the 150
output features split 75/75 (A/B) so every ACT/DVE instruction runs at full
free-dim width (engine time is proportional to free size only; a 22-row
remainder half costs as much as a 128-row one). Host packs activations as
x[600, 12544] fp16, last_c as c[75, 2, 12544], weights as w[120, 750]
(partition-major, contiguous rows so the load descriptor is >=512B).

Per macro of 1024 tokens (tail macro: 256):
  - one x load [120,5,1024], one c load into the output tile's cell slots,
  - per half F: 2x5 matmuls (K=600 as 5x120) into PSUM [75, 1024],
  - sigmoid -> gate, DVE add/mul for cell (in place over last_c), tanh,
    DVE mul for hidden; all [75, 1024]-wide instructions,
  - ONE store DMA of the output tile [75, 4, 1024] (h_A, cell_A, h_B,
    cell_B) via the gpsimd SWDGE queue (keeps HWDGE + SP free for loads).

All transfers have >=512B innermost chunks (full modeled DMA bandwidth);
DMA engine busy ~73.7us/core is the roofline for 2.1KB/token fp16 traffic.
"""

import numpy as np

N_TOKENS = 100000
UNITS = 150
N_CORES = 8
ROWS_PER_CORE = N_TOKENS // N_CORES  # 12500
ROWS_PAD = 12544                     # 12*1024 + 256
MACROS = [1024] * 12 + [256]
TILE = 512                           # matmul free-dim (= one PSUM bank of fp32)
KDIM = 4 * UNITS                     # 600
KCHUNK = 120
N_KCHUNKS = KDIM // KCHUNK           # 5
MHALF = 75                           # feature half (A: 0:75, B: 75:150)

_CACHE = {}
REPS = 1  # timing aid: repeat the whole macro loop (outputs are idempotent)


def _build_bass():
    import concourse.bacc as bacc
    import concourse.mybir as mybir
    import concourse.tile as tile

    fp32 = mybir.dt.float32
    mmdt = mybir.dt.float16
    nc = bacc.Bacc("TRN2", target_bir_lowering=False, debug=False,
                   num_devices=N_CORES)

    x = nc.dram_tensor("x", [KDIM, ROWS_PAD], mmdt, kind="ExternalInput").ap()
    c = nc.dram_tensor("c", [MHALF, 2, ROWS_PAD], mmdt,
                       kind="ExternalInput").ap()
    w = nc.dram_tensor("w", [KCHUNK, N_KCHUNKS * UNITS], mmdt,
                       kind="ExternalInput").ap()
    out = nc.dram_tensor("out", [MHALF, 4, ROWS_PAD], mmdt,
                         kind="ExternalOutput").ap()

    AF = mybir.ActivationFunctionType

    x_r = x.rearrange("(k p) t -> p k t", p=KCHUNK)   # [120, 5, 12544]
    w_r = w.rearrange("p (k d) -> p k d", k=N_KCHUNKS)  # [120, 5, 150]

    with tile.TileContext(nc) as tc:
        with (
            tc.tile_pool(name="wpool", bufs=1) as wpool,
            # DEEP lookahead: loads finish ~20us before the DMA floor is
            # reached; every macro's ACT/DVE chain (incl. the last one)
            # then completes while trailing store traffic keeps the DMA
            # engines busy, so the kernel ends on an already-ready store
            # instead of draining compute serially (ACT total ~56us fits
            # easily under the 73.7us DMA busy window).
            tc.tile_pool(name="xpool", bufs=6) as xpool,
            tc.tile_pool(name="opool", bufs=6) as opool,
            tc.tile_pool(name="gpool", bufs=3) as gpool,
            # per-(half) PSUM tiles (2 banks each), 3 rotating bufs: deep
            # enough that matmuls never wait on sigmoid drain, shallow
            # enough to pace dispatch (coarse whole-macro psum serialized
            # the PE stream and forced p-state re-ramps)
            tc.tile_pool(name="psum", bufs=3, space="PSUM") as psum_pool,
            tc.tile_pool(name="fill", bufs=1) as fill_pool,
            tc.tile_pool(name="fpsum", bufs=1, space="PSUM") as fpsum_pool,
        ):
            w_tile = wpool.tile([KCHUNK, N_KCHUNKS, UNITS], mmdt)
            nc.sync.dma_start(w_tile[:, :, :], w_r[:, :, :])

            # PE p-state warming: the tensor engine drops to a slow clock
            # after any idle period and takes ~3us of continuous execution
            # to ramp back. Real matmul work per macro (~4.3us) is below
            # the DMA pace (~6.0us), so PE would idle every macro and pay
            # the ramp on each restart. Tiny filler matmuls into a scratch
            # PSUM bank bridge the idle gaps so the engine stays at full
            # clock; they read a zeroed scratch tile and depend on nothing.
            fx = fill_pool.tile([KCHUNK, TILE], mmdt)
            nc.vector.memset(fx[:, :], 0.0)

            def pe_filler(n):
                for _ in range(n):
                    fp = fpsum_pool.tile([MHALF, TILE], fp32)
                    nc.tensor.matmul(fp[:, :], lhsT=fx[:, 0:MHALF],
                                     rhs=fx[:, :], start=True, stop=True)

            # cover the initial w/x load latency so the busy streak starts
            # before the first real matmul is dispatched
            pe_filler(24)

            # Uniform tile shapes/tags across macro sizes so pool-buffer
            # rotation (not fresh allocations) gates every macro's loads --
            # otherwise the taper macros at the end would all prefetch at
            # once and rebuild the drain backlog.
            macros = [m for _ in range(REPS) for m in MACROS]
            lo = 0
            for rep_i, msz in enumerate(macros):
                if rep_i > 0 and lo + msz > ROWS_PAD:
                    lo = 0
                hi = lo + msz
                ntile = (msz + TILE - 1) // TILE

                x_tile = xpool.tile([KCHUNK, N_KCHUNKS, 1024], mmdt)
                nc.sync.dma_start(x_tile[:, :, 0:msz], x_r[:, :, lo:hi])

                # o_tile cols: [0]=h_A, [1]=cell_A, [2]=h_B, [3]=cell_B;
                # c loads land in the cell slots.
                o_tile = opool.tile([MHALF, 4, 1024], mmdt)
                nc.sync.dma_start(o_tile[:, 1::2, 0:msz], c[:, :, lo:hi])

                gate = gpool.tile([MHALF, 2, 1024], mmdt)

                for f in range(2):
                    pre = psum_pool.tile([MHALF, 1024], fp32)
                    for t in range(ntile):
                        t0, t1 = t * TILE, min((t + 1) * TILE, msz)
                        for k in range(N_KCHUNKS):
                            nc.tensor.matmul(
                                pre[:, t0:t1],
                                lhsT=w_r_tile_slice(w_tile, k, f),
                                rhs=x_tile[:, k, t0:t1],
                                start=(k == 0),
                                stop=(k == N_KCHUNKS - 1),
                            )
                    nc.scalar.activation(gate[:, f, 0:msz],
                                         pre[:, 0:msz], AF.Sigmoid)
                for f in range(2):
                    cell = o_tile[:, 2 * f + 1, 0:msz]
                    nc.vector.tensor_add(cell, cell, gate[:, f, 0:msz])
                    nc.vector.tensor_mul(cell, gate[:, f, 0:msz], cell)
                for f in range(2):
                    nc.scalar.activation(o_tile[:, 2 * f, 0:msz],
                                         o_tile[:, 2 * f + 1, 0:msz], AF.Tanh)
                for f in range(2):
                    hid = o_tile[:, 2 * f, 0:msz]
                    nc.vector.tensor_mul(hid, gate[:, f, 0:msz], hid)

                nc.gpsimd.dma_start(out[:, :, lo:hi], o_tile[:, :, 0:msz])
                if msz == 1024:
                    pe_filler(2)
                lo = hi

    nc.compile()
    return nc


def w_r_tile_slice(w_tile, k, f):
    return w_tile[:, k, f * MHALF:(f + 1) * MHALF]


def _get_nc():
    if "nc" not in _CACHE:
        _CACHE["nc"] = _build_bass()
    return _CACHE["nc"]


def kernel(s_in, s_out, h_in, h_out, last_c,
           w_in_input, w_out_input, u_in_input, u_out_input):
    from concourse.bass_utils import run_bass_kernel_spmd

    nc = _get_nc()

    f16 = np.float16

    wcat = np.concatenate(
        [w_in_input, w_out_input, u_in_input, u_out_input],
        axis=0).astype(np.float32)                      # [600, 150]
    # w[p, k*150+d] = wcat[k*120+p, d]
    wp = np.ascontiguousarray(
        wcat.reshape(N_KCHUNKS, KCHUNK, UNITS).transpose(1, 0, 2)
        .reshape(KCHUNK, N_KCHUNKS * UNITS)).astype(f16)

    in_maps = []
    for core in range(N_CORES):
        rows = slice(core * ROWS_PER_CORE, (core + 1) * ROWS_PER_CORE)
        xT = np.zeros((KDIM, ROWS_PAD), dtype=f16)
        for j, X in enumerate((s_in, s_out, h_in, h_out)):
            xT[j * UNITS:(j + 1) * UNITS, :ROWS_PER_CORE] = \
                np.asarray(X[rows]).T.astype(f16)
        cp = np.zeros((MHALF, 2, ROWS_PAD), dtype=f16)
        cT = np.asarray(last_c[rows]).T.astype(f16)     # [150, 12500]
        cp[:, 0, :ROWS_PER_CORE] = cT[:MHALF]
        cp[:, 1, :ROWS_PER_CORE] = cT[MHALF:]
        in_maps.append({"x": xT, "c": cp, "w": wp})

    res = run_bass_kernel_spmd(nc, in_maps, core_ids=list(range(N_CORES)))

    hidden = np.empty((N_TOKENS, UNITS), dtype=np.float32)
    cell = np.empty((N_TOKENS, UNITS), dtype=np.float32)
    for core in range(N_CORES):
        rows = slice(core * ROWS_PER_CORE, (core + 1) * ROWS_PER_CORE)
        o = res.results[core]["out"][:, :, :ROWS_PER_CORE]  # [75, 4, 12500]
        hidden[rows, :MHALF] = o[:, 0, :].T
        hidden[rows, MHALF:] = o[:, 2, :].T
        cell[rows, :MHALF] = o[:, 1, :].T
        cell[rows, MHALF:] = o[:, 3, :].T
    return hidden, cell


# revision 16
# speedup vs baseline: 1.0290x; 1.0290x over previous
"""Trainium2 Bass kernel for CustomizeLSTMCell (fused 4-matmul LSTM-like cell).

Math (per token row x of N=100000, H=150):
    pre    = s_in @ W_in + s_out @ W_out + h_in @ U_in + h_out @ U_out
    gate   = sigmoid(pre)
    cell   = gate * last_c + gate * gate = gate * (last_c + gate)
    hidden = gate * tanh(cell)
returns (hidden, cell)

Strategy: data-parallel over tokens across 8 cores (12500 rows/core, padded
to 12544 = 12*1024 + 256). Feature-major (transposed) on chip with the 150
output features split 75/75 (A/B) so every ACT/DVE instruction runs at full
free-dim width (engine time is proportional to free size only; a 22-row
remainder half costs as much as a 128-row one). Host packs activations as
x[600, 12544] fp16, last_c as c[75, 2, 12544], weights as w[120, 750]
(partition-major, contiguous rows so the load descriptor is >=512B).

Per macro of 1024 tokens (tail macro: 256):
  - one x load [120,5,1024], one c load into the output tile's cell slots,
  - per half F: 2x5 matmuls (K=600 as 5x120) into PSUM [75, 1024],
  - sigmoid -> gate, DVE add/mul for cell (in place over last_c), tanh,
    DVE mul for hidden; all [75, 1024]-wide instructions,
  - ONE store DMA of the output tile [75, 4, 1024] (h_A, cell_A, h_B,
    cell_B) via the gpsimd SWDGE queue (keeps HWDGE + SP free for loads).

All transfers have >=512B innermost chunks (full modeled DMA bandwidth);
DMA engine busy ~73.7us/core is the roofline for 2.1KB/token fp16 traffic.
"""

import numpy as np

N_TOKENS = 100000
UNITS = 150
N_CORES = 8
ROWS_PER_CORE = N_TOKENS // N_CORES  # 12500
ROWS_PAD = 12544                     # 12*1024 + 256
MACROS = [1024] * 12 + [256]
TILE = 512                           # matmul free-dim (= one PSUM bank of fp32)
KDIM = 4 * UNITS                     # 600
KCHUNK = 120
N_KCHUNKS = KDIM // KCHUNK           # 5
MHALF = 75                           # feature half (A: 0:75, B: 75:150)

_CACHE = {}
REPS = 1  # timing aid: repeat the whole macro loop (outputs are idempotent)


def _build_bass():
    import concourse.bacc as bacc
    import concourse.mybir as mybir
    import concourse.tile as tile

    fp32 = mybir.dt.float32
    mmdt = mybir.dt.float16
    nc = bacc.Bacc("TRN2", target_bir_lowering=False, debug=False,
                   num_devices=N_CORES)

    x = nc.dram_tensor("x", [KDIM, ROWS_PAD], mmdt, kind="ExternalInput").ap()
    c = nc.dram_tensor("c", [MHALF, 2, ROWS_PAD], mmdt,
                       kind="ExternalInput").ap()
    w = nc.dram_tensor("w", [KCHUNK, N_KCHUNKS * UNITS], mmdt,
                       kind="ExternalInput").ap()
    out = nc.dram_tensor("out", [MHALF, 4, ROWS_PAD], mmdt,
                         kind="ExternalOutput").ap()

    AF = mybir.ActivationFunctionType

    x_r = x.rearrange("(k p) t -> p k t", p=KCHUNK)   # [120, 5, 12544]
    w_r = w.rearrange("p (k d) -> p k d", k=N_KCHUNKS)  # [120, 5, 150]

    with tile.TileContext(nc) as tc:
        with (
            tc.tile_pool(name="wpool", bufs=1) as wpool,
            # DEEP lookahead: loads finish ~20us before the DMA floor is
            # reached; every macro's ACT/DVE chain (incl. the last one)
            # then completes while trailing store traffic keeps the DMA
            # engines busy, so the kernel ends on an already-ready store
            # instead of draining compute serially (ACT total ~56us fits
            # easily under the 73.7us DMA busy window).
            tc.tile_pool(name="xpool", bufs=5) as xpool,
            tc.tile_pool(name="opool", bufs=4) as opool,
            tc.tile_pool(name="odef", bufs=1) as odef_pool,
            tc.tile_pool(name="gpool", bufs=3) as gpool,
            # per-(half) PSUM tiles (2 banks each), 3 rotating bufs: deep
            # enough that matmuls never wait on sigmoid drain, shallow
            # enough to pace dispatch (coarse whole-macro psum serialized
            # the PE stream and forced p-state re-ramps)
            tc.tile_pool(name="psum", bufs=3, space="PSUM") as psum_pool,
            tc.tile_pool(name="fill", bufs=1) as fill_pool,
            tc.tile_pool(name="fpsum", bufs=1, space="PSUM") as fpsum_pool,
        ):
            w_tile = wpool.tile([KCHUNK, N_KCHUNKS, UNITS], mmdt)
            nc.sync.dma_start(w_tile[:, :, :], w_r[:, :, :])

            # PE p-state warming: the tensor engine drops to a slow clock
            # after any idle period and takes ~3us of continuous execution
            # to ramp back. Real matmul work per macro (~4.3us) is below
            # the DMA pace (~6.0us), so PE would idle every macro and pay
            # the ramp on each restart. Tiny filler matmuls into a scratch
            # PSUM bank bridge the idle gaps so the engine stays at full
            # clock; they read a zeroed scratch tile and depend on nothing.
            fx = fill_pool.tile([KCHUNK, TILE], mmdt)
            nc.vector.memset(fx[:, :], 0.0)

            def pe_filler(n):
                for _ in range(n):
                    fp = fpsum_pool.tile([MHALF, TILE], fp32)
                    nc.tensor.matmul(fp[:, :], lhsT=fx[:, 0:MHALF],
                                     rhs=fx[:, :], start=True, stop=True)

            # cover the initial w/x load latency so the busy streak starts
            # before the first real matmul is dispatched
            pe_filler(24)

            # Uniform tile shapes/tags across macro sizes so pool-buffer
            # rotation (not fresh allocations) gates every macro's loads --
            # otherwise the taper macros at the end would all prefetch at
            # once and rebuild the drain backlog.
            # Endgame: after the last load there is ~7us of serial latency
            # (matmul -> sigmoid -> DVE -> tanh -> DVE -> SWDGE gen ->
            # store) that the DMA engines cannot overlap -- any store whose
            # chain already finished has also already transferred. So the
            # first N_DEFER macros' stores are HELD BACK (their o_tiles
            # stay live in dedicated buffers) and issued after the loop on
            # the sync queue: ~8.5us of ready-to-fire trailing transfers
            # that keep the DMA engines busy while the final chains drain.
            N_DEFER = 5
            deferred = []

            macros = [m for _ in range(REPS) for m in MACROS]
            lo = 0
            for rep_i, msz in enumerate(macros):
                if rep_i > 0 and lo + msz > ROWS_PAD:
                    lo = 0
                hi = lo + msz
                ntile = (msz + TILE - 1) // TILE
                defer = rep_i < N_DEFER

                x_tile = xpool.tile([KCHUNK, N_KCHUNKS, 1024], mmdt)
                nc.sync.dma_start(x_tile[:, :, 0:msz], x_r[:, :, lo:hi])

                # o_tile cols: [0]=h_A, [1]=cell_A, [2]=h_B, [3]=cell_B;
                # c loads land in the cell slots.
                if defer:
                    o_tile = odef_pool.tile([MHALF, 4, 1024], mmdt,
                                            tag=f"od{rep_i}")
                else:
                    o_tile = opool.tile([MHALF, 4, 1024], mmdt)
                nc.sync.dma_start(o_tile[:, 1::2, 0:msz], c[:, :, lo:hi])

                gate = gpool.tile([MHALF, 2, 1024], mmdt)

                for f in range(2):
                    pre = psum_pool.tile([MHALF, 1024], fp32)
                    for t in range(ntile):
                        t0, t1 = t * TILE, min((t + 1) * TILE, msz)
                        for k in range(N_KCHUNKS):
                            nc.tensor.matmul(
                                pre[:, t0:t1],
                                lhsT=w_r_tile_slice(w_tile, k, f),
                                rhs=x_tile[:, k, t0:t1],
                                start=(k == 0),
                                stop=(k == N_KCHUNKS - 1),
                            )
                    nc.scalar.activation(gate[:, f, 0:msz],
                                         pre[:, 0:msz], AF.Sigmoid)
                for f in range(2):
                    cell = o_tile[:, 2 * f + 1, 0:msz]
                    nc.vector.tensor_add(cell, cell, gate[:, f, 0:msz])
                    nc.vector.tensor_mul(cell, gate[:, f, 0:msz], cell)
                for f in range(2):
                    nc.scalar.activation(o_tile[:, 2 * f, 0:msz],
                                         o_tile[:, 2 * f + 1, 0:msz], AF.Tanh)
                for f in range(2):
                    hid = o_tile[:, 2 * f, 0:msz]
                    nc.vector.tensor_mul(hid, gate[:, f, 0:msz], hid)

                if defer:
                    deferred.append((o_tile, lo, hi, msz))
                else:
                    nc.gpsimd.dma_start(out[:, :, lo:hi], o_tile[:, :, 0:msz])
                if msz == 1024:
                    pe_filler(8)
                lo = hi

            for o_tile, dlo, dhi, dmsz in deferred:
                nc.sync.dma_start(out[:, :, dlo:dhi], o_tile[:, :, 0:dmsz])

    nc.compile()
    return nc


def w_r_tile_slice(w_tile, k, f):
    return w_tile[:, k, f * MHALF:(f + 1) * MHALF]


def _get_nc():
    if "nc" not in _CACHE:
        _CACHE["nc"] = _build_bass()
    return _CACHE["nc"]


def kernel(s_in, s_out, h_in, h_out, last_c,
           w_in_input, w_out_input, u_in_input, u_out_input):
    from concourse.bass_utils import run_bass_kernel_spmd

    nc = _get_nc()

    f16 = np.float16

    wcat = np.concatenate(
        [w_in_input, w_out_input, u_in_input, u_out_input],
        axis=0).astype(np.float32)                      # [600, 150]
    # w[p, k*150+d] = wcat[k*120+p, d]
    wp = np.ascontiguousarray(
        wcat.reshape(N_KCHUNKS, KCHUNK, UNITS).transpose(1, 0, 2)
        .reshape(KCHUNK, N_KCHUNKS * UNITS)).astype(f16)

    in_maps = []
    for core in range(N_CORES):
        rows = slice(core * ROWS_PER_CORE, (core + 1) * ROWS_PER_CORE)
        xT = np.zeros((KDIM, ROWS_PAD), dtype=f16)
        for j, X in enumerate((s_in, s_out, h_in, h_out)):
            xT[j * UNITS:(j + 1) * UNITS, :ROWS_PER_CORE] = \
                np.asarray(X[rows]).T.astype(f16)
        cp = np.zeros((MHALF, 2, ROWS_PAD), dtype=f16)
        cT = np.asarray(last_c[rows]).T.astype(f16)     # [150, 12500]
        cp[:, 0, :ROWS_PER_CORE] = cT[:MHALF]
        cp[:, 1, :ROWS_PER_CORE] = cT[MHALF:]
        in_maps.append({"x": xT, "c": cp, "w": wp})

    res = run_bass_kernel_spmd(nc, in_maps, core_ids=list(range(N_CORES)))

    hidden = np.empty((N_TOKENS, UNITS), dtype=np.float32)
    cell = np.empty((N_TOKENS, UNITS), dtype=np.float32)
    for core in range(N_CORES):
        rows = slice(core * ROWS_PER_CORE, (core + 1) * ROWS_PER_CORE)
        o = res.results[core]["out"][:, :, :ROWS_PER_CORE]  # [75, 4, 12500]
        hidden[rows, :MHALF] = o[:, 0, :].T
        hidden[rows, MHALF:] = o[:, 2, :].T
        cell[rows, :MHALF] = o[:, 1, :].T
        cell[rows, MHALF:] = o[:, 3, :].T
    return hidden, cell


# revision 18
# speedup vs baseline: 1.1012x; 1.0701x over previous
"""Trainium2 Bass kernel for CustomizeLSTMCell (fused 4-matmul LSTM-like cell).

Math (per token row x of N=100000, H=150):
    pre    = s_in @ W_in + s_out @ W_out + h_in @ U_in + h_out @ U_out
    gate   = sigmoid(pre)
    cell   = gate * last_c + gate * gate = gate * (last_c + gate)
    hidden = gate * tanh(cell)
returns (hidden, cell)

Strategy: data-parallel over tokens across 8 cores (12500 rows/core, padded
to 12544 = 12*1024 + 256). Feature-major (transposed) on chip with the 150
output features split 75/75 (A/B) so every ACT/DVE instruction runs at full
free-dim width (engine time is proportional to free size only; a 22-row
remainder half costs as much as a 128-row one). Host packs activations as
x[600, 12544] fp16, last_c as c[75, 2, 12544], weights as w[120, 750]
(partition-major, contiguous rows so the load descriptor is >=512B).

Per macro of 1024 tokens (tail macro: 256):
  - one x load [120,5,1024], one c load into the output tile's cell slots,
  - per half F: 2x5 matmuls (K=600 as 5x120) into PSUM [75, 1024],
  - sigmoid -> gate, DVE add/mul for cell (in place over last_c), tanh,
    DVE mul for hidden; all [75, 1024]-wide instructions,
  - ONE store DMA of the output tile [75, 4, 1024] (h_A, cell_A, h_B,
    cell_B) via the gpsimd SWDGE queue (keeps HWDGE + SP free for loads).

All transfers have >=512B innermost chunks (full modeled DMA bandwidth);
DMA engine busy ~73.7us/core is the roofline for 2.1KB/token fp16 traffic.
"""

import numpy as np

N_TOKENS = 100000
UNITS = 150
N_CORES = 8
ROWS_PER_CORE = N_TOKENS // N_CORES  # 12500
ROWS_PAD = 12544                     # 12*1024 + 256
MACROS = [1024] * 12 + [256]
TILE = 512                           # matmul free-dim (= one PSUM bank of fp32)
KDIM = 4 * UNITS                     # 600
KCHUNK = 120
N_KCHUNKS = KDIM // KCHUNK           # 5
MHALF = 75                           # feature half (A: 0:75, B: 75:150)

_CACHE = {}
REPS = 1  # timing aid: repeat the whole macro loop (outputs are idempotent)


def _build_bass():
    import concourse.bacc as bacc
    import concourse.mybir as mybir
    import concourse.tile as tile

    fp32 = mybir.dt.float32
    mmdt = mybir.dt.float16
    nc = bacc.Bacc("TRN2", target_bir_lowering=False, debug=False,
                   num_devices=N_CORES)

    x = nc.dram_tensor("x", [KDIM, ROWS_PAD], mmdt, kind="ExternalInput").ap()
    c = nc.dram_tensor("c", [MHALF, 2, ROWS_PAD], mmdt,
                       kind="ExternalInput").ap()
    w = nc.dram_tensor("w", [KCHUNK, N_KCHUNKS * UNITS], mmdt,
                       kind="ExternalInput").ap()
    out = nc.dram_tensor("out", [MHALF, 4, ROWS_PAD], mmdt,
                         kind="ExternalOutput").ap()

    AF = mybir.ActivationFunctionType

    x_r = x.rearrange("(k p) t -> p k t", p=KCHUNK)   # [120, 5, 12544]
    w_r = w.rearrange("p (k d) -> p k d", k=N_KCHUNKS)  # [120, 5, 150]

    with tile.TileContext(nc) as tc:
        with (
            tc.tile_pool(name="wpool", bufs=1) as wpool,
            # DEEP lookahead: loads finish ~20us before the DMA floor is
            # reached; every macro's ACT/DVE chain (incl. the last one)
            # then completes while trailing store traffic keeps the DMA
            # engines busy, so the kernel ends on an already-ready store
            # instead of draining compute serially (ACT total ~56us fits
            # easily under the 73.7us DMA busy window).
            tc.tile_pool(name="xpool", bufs=6) as xpool,
            tc.tile_pool(name="opool", bufs=4) as opool,
            tc.tile_pool(name="odef", bufs=1) as odef_pool,
            tc.tile_pool(name="gpool", bufs=3) as gpool,
            # per-(half) PSUM tiles (2 banks each), 3 rotating bufs: deep
            # enough that matmuls never wait on sigmoid drain, shallow
            # enough to pace dispatch (coarse whole-macro psum serialized
            # the PE stream and forced p-state re-ramps)
            tc.tile_pool(name="psum", bufs=3, space="PSUM") as psum_pool,
            tc.tile_pool(name="fill", bufs=1) as fill_pool,
            tc.tile_pool(name="fpsum", bufs=1, space="PSUM") as fpsum_pool,
        ):
            w_tile = wpool.tile([KCHUNK, N_KCHUNKS, UNITS], mmdt)
            nc.sync.dma_start(w_tile[:, :, :], w_r[:, :, :])

            # PE p-state warming: the tensor engine drops to a slow clock
            # after any idle period and takes ~3us of continuous execution
            # to ramp back. Real matmul work per macro (~4.3us) is below
            # the DMA pace (~6.0us), so PE would idle every macro and pay
            # the ramp on each restart. Tiny filler matmuls into a scratch
            # PSUM bank bridge the idle gaps so the engine stays at full
            # clock; they read a zeroed scratch tile and depend on nothing.
            fx = fill_pool.tile([KCHUNK, TILE], mmdt)
            nc.vector.memset(fx[:, :], 0.0)

            def pe_filler(n):
                for _ in range(n):
                    fp = fpsum_pool.tile([MHALF, TILE], fp32)
                    nc.tensor.matmul(fp[:, :], lhsT=fx[:, 0:MHALF],
                                     rhs=fx[:, :], start=True, stop=True)

            # cover the initial w/x load latency so the busy streak starts
            # before the first real matmul is dispatched
            pe_filler(24)

            # Uniform tile shapes/tags across macro sizes so pool-buffer
            # rotation (not fresh allocations) gates every macro's loads --
            # otherwise the taper macros at the end would all prefetch at
            # once and rebuild the drain backlog.
            # Endgame: after the last load there is ~7us of serial latency
            # (matmul -> sigmoid -> DVE -> tanh -> DVE -> SWDGE gen ->
            # store) that the DMA engines cannot overlap -- any store whose
            # chain already finished has also already transferred. So the
            # first N_DEFER macros' stores are HELD BACK (their o_tiles
            # stay live in dedicated buffers) and issued after the loop on
            # the sync queue: ~8.5us of ready-to-fire trailing transfers
            # that keep the DMA engines busy while the final chains drain.
            N_DEFER = 5
            deferred = []

            macros = [m for _ in range(REPS) for m in MACROS]
            lo = 0
            for rep_i, msz in enumerate(macros):
                if rep_i > 0 and lo + msz > ROWS_PAD:
                    lo = 0
                hi = lo + msz
                ntile = (msz + TILE - 1) // TILE
                defer = rep_i < N_DEFER

                x_tile = xpool.tile([KCHUNK, N_KCHUNKS, 1024], mmdt)
                nc.sync.dma_start(x_tile[:, :, 0:msz], x_r[:, :, lo:hi])

                # o_tile cols: [0]=h_A, [1]=cell_A, [2]=h_B, [3]=cell_B;
                # c loads land in the cell slots.
                if defer:
                    o_tile = odef_pool.tile([MHALF, 4, 1024], mmdt,
                                            tag=f"od{rep_i}")
                else:
                    o_tile = opool.tile([MHALF, 4, 1024], mmdt)
                nc.sync.dma_start(o_tile[:, 1::2, 0:msz], c[:, :, lo:hi])

                gate = gpool.tile([MHALF, 2, 1024], mmdt)

                for f in range(2):
                    pre = psum_pool.tile([MHALF, 1024], fp32)
                    for t in range(ntile):
                        t0, t1 = t * TILE, min((t + 1) * TILE, msz)
                        for k in range(N_KCHUNKS):
                            nc.tensor.matmul(
                                pre[:, t0:t1],
                                lhsT=w_r_tile_slice(w_tile, k, f),
                                rhs=x_tile[:, k, t0:t1],
                                start=(k == 0),
                                stop=(k == N_KCHUNKS - 1),
                            )
                    nc.scalar.activation(gate[:, f, 0:msz],
                                         pre[:, 0:msz], AF.Sigmoid)
                for f in range(2):
                    cell = o_tile[:, 2 * f + 1, 0:msz]
                    nc.vector.tensor_add(cell, cell, gate[:, f, 0:msz])
                    nc.vector.tensor_mul(cell, gate[:, f, 0:msz], cell)
                for f in range(2):
                    nc.scalar.activation(o_tile[:, 2 * f, 0:msz],
                                         o_tile[:, 2 * f + 1, 0:msz], AF.Tanh)
                for f in range(2):
                    hid = o_tile[:, 2 * f, 0:msz]
                    nc.vector.tensor_mul(hid, gate[:, f, 0:msz], hid)

                if defer:
                    deferred.append((o_tile, lo, hi, msz))
                else:
                    nc.gpsimd.dma_start(out[:, :, lo:hi], o_tile[:, :, 0:msz])
                # filler count tracks the DMA cadence of each phase so PE
                # neither idles (p-state reset) nor backlogs (late chains):
                # deferred-store phase has loads only (4.27us/macro ~= real
                # matmul work), steady phase is 5.97us/macro, and the last
                # macros run bare so PE finishes early and the final chains
                # hide under the deferred trailing stores.
                n_macros = len(macros)
                if defer:
                    pe_filler(1)
                elif rep_i < n_macros - 1 and msz == 1024:
                    pe_filler(8)
                lo = hi

            for o_tile, dlo, dhi, dmsz in deferred:
                nc.sync.dma_start(out[:, :, dlo:dhi], o_tile[:, :, 0:dmsz])

    nc.compile()
    return nc


def w_r_tile_slice(w_tile, k, f):
    return w_tile[:, k, f * MHALF:(f + 1) * MHALF]


def _get_nc():
    if "nc" not in _CACHE:
        _CACHE["nc"] = _build_bass()
    return _CACHE["nc"]


def kernel(s_in, s_out, h_in, h_out, last_c,
           w_in_input, w_out_input, u_in_input, u_out_input):
    from concourse.bass_utils import run_bass_kernel_spmd

    nc = _get_nc()

    f16 = np.float16

    wcat = np.concatenate(
        [w_in_input, w_out_input, u_in_input, u_out_input],
        axis=0).astype(np.float32)                      # [600, 150]
    # w[p, k*150+d] = wcat[k*120+p, d]
    wp = np.ascontiguousarray(
        wcat.reshape(N_KCHUNKS, KCHUNK, UNITS).transpose(1, 0, 2)
        .reshape(KCHUNK, N_KCHUNKS * UNITS)).astype(f16)

    in_maps = []
    for core in range(N_CORES):
        rows = slice(core * ROWS_PER_CORE, (core + 1) * ROWS_PER_CORE)
        xT = np.zeros((KDIM, ROWS_PAD), dtype=f16)
        for j, X in enumerate((s_in, s_out, h_in, h_out)):
            xT[j * UNITS:(j + 1) * UNITS, :ROWS_PER_CORE] = \
                np.asarray(X[rows]).T.astype(f16)
        cp = np.zeros((MHALF, 2, ROWS_PAD), dtype=f16)
        cT = np.asarray(last_c[rows]).T.astype(f16)     # [150, 12500]
        cp[:, 0, :ROWS_PER_CORE] = cT[:MHALF]
        cp[:, 1, :ROWS_PER_CORE] = cT[MHALF:]
        in_maps.append({"x": xT, "c": cp, "w": wp})

    res = run_bass_kernel_spmd(nc, in_maps, core_ids=list(range(N_CORES)))

    hidden = np.empty((N_TOKENS, UNITS), dtype=np.float32)
    cell = np.empty((N_TOKENS, UNITS), dtype=np.float32)
    for core in range(N_CORES):
        rows = slice(core * ROWS_PER_CORE, (core + 1) * ROWS_PER_CORE)
        o = res.results[core]["out"][:, :, :ROWS_PER_CORE]  # [75, 4, 12500]
        hidden[rows, :MHALF] = o[:, 0, :].T
        hidden[rows, MHALF:] = o[:, 2, :].T
        cell[rows, :MHALF] = o[:, 1, :].T
        cell[rows, MHALF:] = o[:, 3, :].T
    return hidden, cell
